# revision 1
# baseline (speedup 1.0000x reference)
"""Trainium2 Bass kernel for nn_DecodingLoss (cepstrum decoding loss).

Math (per 4096-sample window):
  cep = irfft(log(|rfft(x)| + eps))[DELAYS]; softargmax(beta=1e10) -> argmax idx;
  loss = clip(|idx - symbol|,0,1); per-audio sums -> 5 scalar outputs.

Kernel strategy (8 cores, pure data parallel over the batch dim):
  FFT 4096 = 32 x 128 Cooley-Tukey: n = 128*t + s  (t<32, s<128)
    stage1 (PE): A[u,s] = sum_t x[128t+s] W32^{tu}  - block-diag(4 windows) C32/S32
    corner turn (PE transpose)
    stage2 (PE): X[k=u+32v] = sum_s A[u,s] W4096^{s(u+32v)}  - twiddle folded into
      per-u stationary H_u[s,v]; only k=1..2048 computed (hermitian; k=0 dropped -
      a uniform shift of all cep taps cancels in softmax exactly).
  log|X|: L = 0.5*log(Xre^2+Xim^2+1e-10) (ACT), then cep taps via one PE projection
  (delays are multiples of 32 -> cos tables fold), softargmax + loss on DVE/ACT.
  Host: sums per-audio errors and mirrors the reference's final scalar math.
"""
import numpy as np
import ml_dtypes

import concourse.bass as bass
import concourse.mybir as mybir
from concourse import tile
from concourse.bass_utils import run_bass_kernel_spmd

FP32 = mybir.dt.float32
F32R = mybir.dt.float32r
BF16 = mybir.dt.bfloat16
I32 = mybir.dt.int32

B, NW, WIN = 64, 128, 4096
NCORES = 8
BLOC = B // NCORES              # 8 audio rows per core
WLOC = BLOC * NW                # 1024 windows per core
T, S, U = 32, 128, 32           # n = 128 t + s ; k = u + 32 v
NV = 64                         # v-grid size per u
ITERS = 4
WPI = WLOC // ITERS             # 256 windows per iteration
G = WPI // 4                    # 64 groups of 4 windows
DELAYS = np.array([64, 96, 128, 160, 192, 224, 256, 288])
BETA = 1e10

_cache = {}


def _hoist_waits(bir_json):
    """This walrus build rejects instructions carrying attached semaphore waits
    ("Too many sync wait commands"); raw-bass style standalone EventSemaphore
    waits compile and run. Hoist every attached wait into its own
    EventSemaphore on the same engine queue; updates stay attached."""
    import json
    d = json.loads(bir_json)
    n = 0
    for fn in d["functions"]:
        for bb in fn["blocks"]:
            out = []
            for ins in bb["instructions"]:
                si = ins.get("sync_info")
                waits = (si or {}).get("on_wait") or []
                if waits and ins.get("opcode") != "EventSemaphore" and ins.get("engine"):
                    for w in waits:
                        n += 1
                        out.append({
                            "name": f"hoistw-{n}", "opcode": "EventSemaphore",
                            "engine": ins["engine"], "ins": [], "outs": [],
                            "sync_info": {"on_wait": [w], "on_update": []},
                        })
                    si["on_wait"] = []
                out.append(ins)
            bb["instructions"] = out
    return json.dumps(d).encode()


def _install_hoist(nc):
    orig = nc.to_json_bytes
    nc.to_json_bytes = lambda: _hoist_waits(orig())
    return nc
LINEARIZE = False


def _tables():
    t = np.arange(T)[:, None]
    u = np.arange(U)[None, :]
    c32 = np.cos(2 * np.pi * t * u / 32.0)
    s32n = -np.sin(2 * np.pi * t * u / 32.0)
    bdc = np.zeros((128, 128), np.float64)
    bds = np.zeros((128, 128), np.float64)
    for w in range(4):
        bdc[w * 32:w * 32 + 32, w * 32:w * 32 + 32] = c32
        bds[w * 32:w * 32 + 32, w * 32:w * 32 + 32] = s32n

    # k-grid per u: u==0 -> k = 32*(j+1) (j=0..63), else k = u + 32*j
    kgrid = np.zeros((U, NV), np.int64)
    kgrid[0] = 32 * (np.arange(NV) + 1)
    for uu in range(1, U):
        kgrid[uu] = uu + 32 * np.arange(NV)

    s = np.arange(S)[:, None]
    h2 = np.zeros((S, U, 3, NV), np.float64)
    for uu in range(U):
        ph = 2 * np.pi * s * kgrid[uu][None, :] / 4096.0
        h2[:, uu, 0] = np.cos(ph)           # Hre
        h2[:, uu, 1] = -np.sin(ph)          # Him
        h2[:, uu, 2] = np.sin(ph)           # -Him
    # projection: cep[d] = sum_k wk*0.5*log(m2)[k]*cos(2 pi k d/4096)/4096
    pp = np.zeros((128, 16, 8), np.float64)
    for p in range(16):
        for half in range(2):
            uu = 2 * p + half
            k = kgrid[uu]
            wk = np.where(k == 2048, 1.0, 2.0)
            for j, d in enumerate(DELAYS):
                pp[half * 64:half * 64 + 64, p, j] = (
                    wk * 0.5 * np.cos(2 * np.pi * k * d / 4096.0) / 4096.0)
    ident = np.eye(128)
    idxt = np.broadcast_to(np.arange(8.0), (128, 8)).copy()
    return (bdc.astype(ml_dtypes.bfloat16), bds.astype(ml_dtypes.bfloat16),
            h2.astype(ml_dtypes.bfloat16), pp.astype(np.float32),
            ident.astype(ml_dtypes.bfloat16), idxt.astype(np.float32))


def _build():
    nc = bass.Bass()
    audio = nc.dram_tensor("audio", [WLOC, WIN], BF16, kind="ExternalInput")
    syms = nc.dram_tensor("syms", [WLOC], I32, kind="ExternalInput")
    bdc_d = nc.dram_tensor("bdc", [128, 128], BF16, kind="ExternalInput")
    bds_d = nc.dram_tensor("bds", [128, 128], BF16, kind="ExternalInput")
    h2_d = nc.dram_tensor("h2", [S, U, 3, NV], BF16, kind="ExternalInput")
    pp_d = nc.dram_tensor("pp", [128, 16, 8], F32R, kind="ExternalInput")
    id_d = nc.dram_tensor("ident", [128, 128], BF16, kind="ExternalInput")
    ix_d = nc.dram_tensor("idxt", [128, 8], FP32, kind="ExternalInput")
    idf_d = nc.dram_tensor("identf", [128, 128], FP32, kind="ExternalInput")
    loss_out = nc.dram_tensor("loss_out", [WLOC], FP32, kind="ExternalOutput")

    with tile.TileContext(nc, linearize=LINEARIZE) as tc:
        with (
            tc.tile_pool(name="consts", bufs=1) as consts,
            tc.tile_pool(name="xt", bufs=2) as xt_pool,
            tc.tile_pool(name="as_", bufs=4) as as_pool,
            tc.tile_pool(name="at", bufs=2) as at_pool,
            tc.tile_pool(name="sq", bufs=2) as sq_pool,
            tc.tile_pool(name="m2", bufs=2) as m2_pool,
            tc.tile_pool(name="lg", bufs=2) as lg_pool,
            tc.tile_pool(name="fin", bufs=2) as fin_pool,
            tc.tile_pool(name="psA", bufs=2, space="PSUM") as psA_pool,
            tc.tile_pool(name="psT", bufs=2, space="PSUM") as psT_pool,
            tc.tile_pool(name="psX", bufs=2, space="PSUM") as psX_pool,
            tc.tile_pool(name="cep", bufs=1, space="PSUM") as cep_pool,
            tc.tile_pool(name="psC", bufs=1, space="PSUM") as psC_pool,
        ):
            bdc = consts.tile([128, 128], BF16, tag="bdc")
            nc.sync.dma_start(bdc[:], bdc_d[:])
            bds = consts.tile([128, 128], BF16, tag="bds")
            nc.sync.dma_start(bds[:], bds_d[:])
            ident = consts.tile([128, 128], BF16, tag="ident")
            nc.sync.dma_start(ident[:], id_d[:])
            idxt = consts.tile([128, 8], FP32, tag="idxt")
            nc.sync.dma_start(idxt[:], ix_d[:])
            identf = consts.tile([128, 128], FP32, tag="identf")
            nc.sync.dma_start(identf[:], idf_d[:])
            h2 = consts.tile([128, U * 3 * NV], BF16, tag="h2")
            nc.sync.dma_start(h2[:], h2_d[:].rearrange("s u c j -> s (u c j)"))
            ppj = consts.tile([128, 128], F32R, tag="ppj")
            nc.sync.dma_start(ppj[:], pp_d[:].rearrange("s p j -> s (p j)"))
            epsb = consts.tile([128, 1], FP32, tag="epsb")
            nc.vector.memset(epsb[:], 1e-10)
            symt = consts.tile([128, BLOC], I32, tag="symt")
            nc.sync.dma_start(symt[:], syms[:].rearrange("(c i) -> i c", i=128))

            def h2c(uu, comp):  # stationary slice for stage-2
                off = uu * (3 * NV) + comp * NV
                return h2[:, off:off + NV]

            for it in range(ITERS):
                xt = xt_pool.tile([128, WPI * 32], BF16, tag="xt")
                nc.sync.dma_start(
                    xt[:].rearrange("p (g s) -> p g s", s=S),
                    audio[it * WPI:(it + 1) * WPI, :]
                    .rearrange("(g w4) (t s) -> (w4 t) g s", w4=4, s=S))

                at_re = at_pool.tile([128, G * 128], BF16, tag="at_re")
                at_im = at_pool.tile([128, G * 128], BF16, tag="at_im")

                for gp in range(G // 2):   # 2 windows-groups (8 windows) per bank
                    psA = psA_pool.tile([128, 512], FP32, tag="psA")
                    x0 = xt[:, gp * 256:gp * 256 + 128]
                    x1 = xt[:, gp * 256 + 128:gp * 256 + 256]
                    nc.tensor.matmul(psA[:, 0:128], bdc[:], x0, start=True, stop=True)
                    nc.tensor.matmul(psA[:, 256:384], bdc[:], x1, start=True, stop=True)
                    nc.tensor.matmul(psA[:, 128:256], bds[:], x0, start=True, stop=True)
                    nc.tensor.matmul(psA[:, 384:512], bds[:], x1, start=True, stop=True)

                    asb = as_pool.tile([128, 512], BF16, tag="asb")
                    if gp % 2 == 0:
                        nc.vector.tensor_copy(asb[:], psA[:])
                    else:
                        nc.scalar.activation(asb[:], psA[:],
                                             mybir.ActivationFunctionType.Copy)

                    psT = psT_pool.tile([128, 512], BF16, tag="psT")
                    for j in range(4):
                        nc.tensor.transpose(psT[:, j * 128:(j + 1) * 128],
                                            asb[:, j * 128:(j + 1) * 128], ident[:])
                    # psT blocks: [Atre_g, Atim_g, Atre_g', Atim_g']
                    g0 = 2 * gp
                    dst_re = at_re[:, g0 * 128:(g0 + 2) * 128].rearrange(
                        "s (g c) -> s g c", g=2)
                    src_re = psT[:].rearrange("s (g x c) -> s g x c", g=2, x=2)[:, :, 0, :]
                    dst_im = at_im[:, g0 * 128:(g0 + 2) * 128].rearrange(
                        "s (g c) -> s g c", g=2)
                    src_im = psT[:].rearrange("s (g x c) -> s g x c", g=2, x=2)[:, :, 1, :]
                    if gp % 2 == 0:
                        nc.scalar.activation(dst_re, src_re,
                                             mybir.ActivationFunctionType.Copy)
                        nc.vector.tensor_copy(dst_im, src_im)
                    else:
                        nc.vector.tensor_copy(dst_re, src_re)
                        nc.scalar.activation(dst_im, src_im,
                                             mybir.ActivationFunctionType.Copy)

                cep = cep_pool.tile([128, 256], FP32, tag="cep")
                for p in range(16):
                    psX = psX_pool.tile([128, 512], FP32, tag="psX")
                    for half in range(2):
                        uu = 2 * p + half
                        ro = slice(half * 64, half * 64 + 64)
                        # moving operand: columns {g*128 + w4*32 + uu}
                        rre = at_re[:].rearrange("s (g w4 u) -> s g w4 u",
                                                 g=G, w4=4)[:, :, :, uu]
                        rim = at_im[:].rearrange("s (g w4 u) -> s g w4 u",
                                                 g=G, w4=4)[:, :, :, uu]
                        nc.tensor.matmul(psX[ro, 0:256], h2c(uu, 0), rre,
                                         start=True, stop=False)
                        nc.tensor.matmul(psX[ro, 0:256], h2c(uu, 2), rim,
                                         start=False, stop=True)
                        nc.tensor.matmul(psX[ro, 256:512], h2c(uu, 1), rre,
                                         start=True, stop=False)
                        nc.tensor.matmul(psX[ro, 256:512], h2c(uu, 0), rim,
                                         start=False, stop=True)
                    sq = sq_pool.tile([128, 512], FP32, tag="sq")
                    nc.scalar.activation(sq[:], psX[:],
                                         mybir.ActivationFunctionType.Square)
                    m2 = m2_pool.tile([128, 256], FP32, tag="m2")
                    nc.vector.tensor_add(m2[:], sq[:, 0:256], sq[:, 256:512])
                    lg = lg_pool.tile([128, 256], F32R, tag="lg")
                    nc.scalar.activation(lg[:], m2[:],
                                         mybir.ActivationFunctionType.Ln,
                                         bias=epsb[:])
                    nc.tensor.matmul(cep[0:8, :], ppj[:, p * 8:(p + 1) * 8], lg[:],
                                     start=(p == 0), stop=(p == 15))

                cep_sb = fin_pool.tile([8, 256], FP32, tag="cep_sb")
                nc.scalar.activation(cep_sb[:], cep[0:8, :],
                                     mybir.ActivationFunctionType.Copy)
                for c in range(2):
                    gc = it * 2 + c
                    psC = psC_pool.tile([128, 8], FP32, tag="psC")
                    nc.tensor.transpose(psC[:], cep_sb[:, c * 128:(c + 1) * 128],
                                        identf[0:8, 0:8])
                    mx = fin_pool.tile([128, 1], FP32, tag="mx")
                    nc.vector.reduce_max(mx[:], psC[:], axis=mybir.AxisListType.X)
                    nb = fin_pool.tile([128, 1], FP32, tag="nb")
                    nc.vector.tensor_scalar_mul(nb[:], mx[:], -BETA)
                    ex = fin_pool.tile([128, 8], FP32, tag="ex")
                    nc.scalar.activation(ex[:], psC[:],
                                         mybir.ActivationFunctionType.Exp,
                                         bias=nb[:], scale=BETA)
                    den = fin_pool.tile([128, 1], FP32, tag="den")
                    nc.vector.reduce_sum(den[:], ex[:], axis=mybir.AxisListType.X)
                    en = fin_pool.tile([128, 8], FP32, tag="en")
                    nc.vector.tensor_mul(en[:], ex[:], idxt[:])
                    num = fin_pool.tile([128, 1], FP32, tag="num")
                    nc.vector.reduce_sum(num[:], en[:], axis=mybir.AxisListType.X)
                    rden = fin_pool.tile([128, 1], FP32, tag="rden")
                    nc.vector.reciprocal(rden[:], den[:])
                    mv = fin_pool.tile([128, 1], FP32, tag="mv")
                    nc.vector.tensor_mul(mv[:], num[:], rden[:])
                    symf = fin_pool.tile([128, 1], FP32, tag="symf")
                    nc.vector.tensor_copy(symf[:], symt[:, gc:gc + 1])
                    df = fin_pool.tile([128, 1], FP32, tag="df")
                    nc.vector.tensor_sub(df[:], mv[:], symf[:])
                    ab = fin_pool.tile([128, 1], FP32, tag="ab")
                    nc.scalar.activation(ab[:], df[:],
                                         mybir.ActivationFunctionType.Abs)
                    ls = fin_pool.tile([128, 1], FP32, tag="ls")
                    nc.vector.tensor_scalar_min(ls[:], ab[:], 1.0)
                    nc.sync.dma_start(
                        loss_out[gc * 128:(gc + 1) * 128], ls[:, 0])
    return nc


def kernel(audio_batch, symbols_batch, num_errs_no_reverb_batch,
           num_errs_reverb_batch):
    audio_batch = np.asarray(audio_batch)
    symbols_batch = np.asarray(symbols_batch, dtype=np.int32)
    nn_ = np.asarray(num_errs_no_reverb_batch).astype(np.float32)
    nr_ = np.asarray(num_errs_reverb_batch).astype(np.float32)

    if "nc" not in _cache:
        _cache["nc"] = _install_hoist(_build())
        _cache["tabs"] = _tables()
    nc = _cache["nc"]
    bdc, bds, h2, pp, ident, idxt = _cache["tabs"]

    audio_bf = (audio_batch.reshape(B, NW * WIN)
                .astype(ml_dtypes.bfloat16)
                .reshape(NCORES, WLOC, WIN))
    syms = symbols_batch.reshape(NCORES, WLOC)
    in_maps = []
    for c in range(NCORES):
        in_maps.append({
            "audio": audio_bf[c], "syms": syms[c],
            "bdc": bdc, "bds": bds, "h2": h2, "pp": pp,
            "ident": ident, "idxt": idxt,
            "identf": np.asarray(ident, dtype=np.float32),
        })
    import os
    res = run_bass_kernel_spmd(nc, in_maps, core_ids=list(range(NCORES)),
                               trace=bool(os.environ.get("KTRACE")))
    _cache["last_res"] = res
    loss = np.concatenate([res.results[c]["loss_out"] for c in range(NCORES)])
    errs = loss.reshape(B, NW).sum(axis=1, dtype=np.float32)

    tot = np.float32(errs.sum())
    diff = nr_ - nn_
    inv_red = np.where(diff == 0, np.float32(1.0), diff / (nr_ - errs))
    ter = np.float32(inv_red.sum())
    denom = np.float32(B * NW)
    return (np.float32(tot / denom), tot, np.float32(ter / B),
            np.float32(nn_.sum() / denom), np.float32(nr_.sum() / denom))



# revision 10
# speedup vs baseline: 1.7248x; 1.7248x over previous
"""Trainium2 Bass kernel for nn_DecodingLoss (cepstrum decoding loss).

Math (per 4096-sample window):
  cep = irfft(log(|rfft(x)| + eps))[DELAYS]; softargmax(beta=1e10) -> argmax idx;
  loss = clip(|idx - symbol|,0,1); per-audio sums -> 5 scalar outputs.

Kernel strategy (8 cores, pure data parallel over the batch dim):
  FFT 4096 = 32 x 128 Cooley-Tukey: n = 128*t + s  (t<32, s<128)
    stage1 (PE): At[s, (u w4)] = sum_t x[128t+s] W32^{tu} computed with the
      window DATA as the stationary operand and the block-diag C32/S32 as the
      moving operand, so the result lands already corner-turned (no PE
      transposes, no extra PSUM->SBUF round trip).
    evac (DVE/Pool): scatter At re/im from per-group PSUM banks into u-major
      SBUF layout [s, u, w] so stage-2 moving operands are contiguous.
    stage2 (PE): X[k=u+32v] for k=1..2048 via per-u stationary
      [cos|-sin] / [sin|cos] pairs (128 out rows = Xre(v) rows 0:64,
      Xim(v) rows 64:128) streaming 256 contiguous window columns.
  |X|^2: ACT Square (bf16 out) then DVE/Pool partition-folding adds;
  log: ACT Ln; cep taps via one PE projection (f32r); softargmax + loss on
  DVE/ACT. Host sums per-audio errors and mirrors the reference scalar math.
"""
import numpy as np
import ml_dtypes

import concourse.bass as bass
import concourse.mybir as mybir
from concourse import tile
from concourse.bass_utils import run_bass_kernel_spmd

FP32 = mybir.dt.float32
F32R = mybir.dt.float32r
BF16 = mybir.dt.bfloat16
I32 = mybir.dt.int32

B, NW, WIN = 64, 128, 4096
NCORES = 8
BLOC = B // NCORES              # 8 audio rows per core
WLOC = BLOC * NW                # 1024 windows per core
T, S, U = 32, 128, 32           # n = 128 t + s ; k = u + 32 v
NV = 64                         # v-grid size per u
ITERS = 4
WPI = WLOC // ITERS             # 256 windows per iteration
G = WPI // 4                    # 64 groups of 4 windows
DELAYS = np.array([64, 96, 128, 160, 192, 224, 256, 288])
BETA = 1e10

_cache = {}


def _hoist_waits(bir_json):
    """This walrus build rejects instructions carrying attached semaphore waits
    ("Too many sync wait commands"); raw-bass style standalone EventSemaphore
    waits compile and run. Hoist every attached wait into its own
    EventSemaphore on the same engine queue; updates stay attached."""
    import json
    d = json.loads(bir_json)
    n = 0
    for fn in d["functions"]:
        for bb in fn["blocks"]:
            out = []
            for ins in bb["instructions"]:
                si = ins.get("sync_info")
                waits = (si or {}).get("on_wait") or []
                if waits and ins.get("opcode") != "EventSemaphore" and ins.get("engine"):
                    for w in waits:
                        n += 1
                        out.append({
                            "name": f"hoistw-{n}", "opcode": "EventSemaphore",
                            "engine": ins["engine"], "ins": [], "outs": [],
                            "sync_info": {"on_wait": [w], "on_update": []},
                        })
                    si["on_wait"] = []
                out.append(ins)
            bb["instructions"] = out
    return json.dumps(d).encode()


def _install_hoist(nc):
    orig = nc.to_json_bytes
    nc.to_json_bytes = lambda: _hoist_waits(orig())
    return nc
LINEARIZE = False


def _tables():
    t = np.arange(T)[:, None]
    u = np.arange(U)[None, :]
    c32 = np.cos(2 * np.pi * t * u / 32.0)          # [t, u]
    s32n = -np.sin(2 * np.pi * t * u / 32.0)

    # moving operand for swapped stage-1: cols = (cs, u, w4'); rows = (w4, t)
    bdcs = np.zeros((4, T, 2, U, 4), np.float64)
    for w4 in range(4):
        bdcs[w4, :, 0, :, w4] = c32
        bdcs[w4, :, 1, :, w4] = s32n
    bdcs = bdcs.reshape(128, 256)

    # k-grid per u: u==0 -> k = 32*(j+1) (j=0..63), else k = u + 32*j
    kgrid = np.zeros((U, NV), np.int64)
    kgrid[0] = 32 * (np.arange(NV) + 1)
    for uu in range(1, U):
        kgrid[uu] = uu + 32 * np.arange(NV)

    s = np.arange(S)[:, None]
    # stage-2 stationaries: per (u, m) a [S, 128] block; out rows 0:64 = Xre(v),
    # rows 64:128 = Xim(v).  m=0 applies to At_re, m=1 to At_im.
    h2m = np.zeros((S, U, 2, 2, NV), np.float64)
    for uu in range(U):
        ph = 2 * np.pi * s * kgrid[uu][None, :] / 4096.0
        h2m[:, uu, 0, 0] = np.cos(ph)
        h2m[:, uu, 0, 1] = -np.sin(ph)
        h2m[:, uu, 1, 0] = np.sin(ph)
        h2m[:, uu, 1, 1] = np.cos(ph)
    h2m = h2m.reshape(S, U * 2 * 128)

    # projection: cep[d] = sum_k wk*0.5*log(m2)[k]*cos(2 pi k d/4096)/4096
    pp = np.zeros((128, 16, 8), np.float64)
    for p in range(16):
        for half in range(2):
            uu = 2 * p + half
            k = kgrid[uu]
            wk = np.where(k == 2048, 1.0, 2.0)
            for j, d in enumerate(DELAYS):
                pp[half * 64:half * 64 + 64, p, j] = (
                    wk * 0.5 * np.cos(2 * np.pi * k * d / 4096.0) / 4096.0)
    idxt = np.broadcast_to(np.arange(8.0), (128, 8)).copy()
    identf = np.eye(128, dtype=np.float32)
    return (bdcs.astype(ml_dtypes.bfloat16), h2m.astype(ml_dtypes.bfloat16),
            pp.astype(np.float32), idxt.astype(np.float32), identf)


def _build():
    nc = bass.Bass()
    audio = nc.dram_tensor("audio", [WLOC, WIN], BF16, kind="ExternalInput")
    syms = nc.dram_tensor("syms", [WLOC], I32, kind="ExternalInput")
    bdcs_d = nc.dram_tensor("bdcs", [128, 256], BF16, kind="ExternalInput")
    h2m_d = nc.dram_tensor("h2m", [S, U * 2 * 128], BF16, kind="ExternalInput")
    pp_d = nc.dram_tensor("pp", [128, 16, 8], F32R, kind="ExternalInput")
    ix_d = nc.dram_tensor("idxt", [128, 8], FP32, kind="ExternalInput")
    idf_d = nc.dram_tensor("identf", [128, 128], FP32, kind="ExternalInput")
    loss_out = nc.dram_tensor("loss_out", [WLOC], FP32, kind="ExternalOutput")

    with tile.TileContext(nc, linearize=LINEARIZE) as tc:
        with (
            tc.tile_pool(name="consts", bufs=1) as consts,
            tc.tile_pool(name="xt", bufs=2) as xt_pool,
            tc.tile_pool(name="at", bufs=2) as at_pool,
            tc.tile_pool(name="sq", bufs=2) as sq_pool,
            tc.tile_pool(name="m2", bufs=2) as m2_pool,
            tc.tile_pool(name="lg", bufs=2) as lg_pool,
            tc.tile_pool(name="fin", bufs=2) as fin_pool,
            tc.tile_pool(name="ps1", bufs=2, space="PSUM") as ps1_pool,
            tc.tile_pool(name="psX", bufs=2, space="PSUM") as psX_pool,
            tc.tile_pool(name="cep", bufs=1, space="PSUM") as cep_pool,
            tc.tile_pool(name="psC", bufs=1, space="PSUM") as psC_pool,
        ):
            bdcs = consts.tile([128, 256], BF16, tag="bdcs")
            nc.sync.dma_start(bdcs[:], bdcs_d[:])
            idxt = consts.tile([128, 8], FP32, tag="idxt")
            nc.sync.dma_start(idxt[:], ix_d[:])
            identf = consts.tile([128, 128], FP32, tag="identf")
            nc.sync.dma_start(identf[:], idf_d[:])
            h2m = consts.tile([128, U * 2 * 128], BF16, tag="h2m")
            nc.sync.dma_start(h2m[:], h2m_d[:])
            ppj = consts.tile([128, 128], F32R, tag="ppj")
            nc.sync.dma_start(ppj[:], pp_d[:].rearrange("s p j -> s (p j)"))
            epsb = consts.tile([128, 1], FP32, tag="epsb")
            nc.vector.memset(epsb[:], 1e-10)
            symt = consts.tile([128, BLOC], I32, tag="symt")
            nc.sync.dma_start(symt[:], syms[:].rearrange("(c i) -> i c", i=128))

            for it in range(ITERS):
                xt = xt_pool.tile([128, WPI * 32], BF16, tag="xt")
                nc.sync.dma_start(
                    xt[:].rearrange("p (g s) -> p g s", s=S),
                    audio[it * WPI:(it + 1) * WPI, :]
                    .rearrange("(g w4) (t s) -> (w4 t) g s", w4=4, s=S))

                # u-major At layout: cols = u*256 + g*4 + w4
                at_re = at_pool.tile([128, G * 128], BF16, tag="at_re")
                at_im = at_pool.tile([128, G * 128], BF16, tag="at_im")
                atv_re = at_re[:].rearrange("s (u g w) -> s g u w", u=U, w=4)
                atv_im = at_im[:].rearrange("s (u g w) -> s g u w", u=U, w=4)

                for b in range(G // 4):   # 4 window-groups (16 windows) per 2 banks
                    ps1 = ps1_pool.tile([128, 1024], FP32, tag="ps1")
                    for g4 in range(4):
                        nc.tensor.matmul(ps1[:, g4 * 256:(g4 + 1) * 256],
                                         xt[:, (4 * b + g4) * 128:
                                            (4 * b + g4 + 1) * 128],
                                         bdcs[:], start=True, stop=True)
                    ps1v = ps1[:].rearrange("s (g4 c u w) -> s g4 c u w",
                                            g4=4, c=2, w=4)
                    nc.vector.tensor_copy(atv_re[:, 4 * b:4 * b + 4],
                                          ps1v[:, :, 0])
                    if b % 2 == 0:
                        nc.vector.tensor_copy(atv_im[:, 4 * b:4 * b + 4],
                                              ps1v[:, :, 1])
                    else:
                        nc.scalar.activation(atv_im[:, 4 * b:4 * b + 4],
                                             ps1v[:, :, 1],
                                             mybir.ActivationFunctionType.Copy)

                cep = cep_pool.tile([128, 256], FP32, tag="cep")
                for p in range(16):
                    psX = psX_pool.tile([128, 512], FP32, tag="psX")
                    for half in range(2):
                        uu = 2 * p + half
                        off = half * 256
                        rre = at_re[:, uu * 256:(uu + 1) * 256]
                        rim = at_im[:, uu * 256:(uu + 1) * 256]
                        st0 = h2m[:, (uu * 2) * 128:(uu * 2 + 1) * 128]
                        st1 = h2m[:, (uu * 2 + 1) * 128:(uu * 2 + 2) * 128]
                        nc.tensor.matmul(psX[:, off:off + 256], st0, rre,
                                         start=True, stop=False)
                        nc.tensor.matmul(psX[:, off:off + 256], st1, rim,
                                         start=False, stop=True)
                    sq = sq_pool.tile([128, 512], BF16, tag="sq")
                    nc.scalar.activation(sq[:], psX[:],
                                         mybir.ActivationFunctionType.Square)
                    # TensorTensor inputs must share a start partition; shift
                    # the cross-partition halves with copies first (GpSimd
                    # can't access PSUM and its tensor ops are slow Q7
                    # software, so everything stays on DVE here).
                    shf = sq_pool.tile([128, 256], BF16, tag="shf")
                    m2 = m2_pool.tile([128, 256], BF16, tag="m2")
                    nc.vector.tensor_copy(shf[0:64, :], sq[64:128, 0:256])
                    nc.vector.tensor_copy(shf[64:128, :], sq[0:64, 256:512])
                    nc.vector.tensor_add(m2[0:64, :], sq[0:64, 0:256],
                                         shf[0:64, :])
                    nc.vector.tensor_add(m2[64:128, :], sq[64:128, 256:512],
                                         shf[64:128, :])
                    lg = lg_pool.tile([128, 256], F32R, tag="lg")
                    nc.scalar.activation(lg[:], m2[:],
                                         mybir.ActivationFunctionType.Ln,
                                         bias=epsb[:])
                    nc.tensor.matmul(cep[0:8, :], ppj[:, p * 8:(p + 1) * 8],
                                     lg[:], start=(p == 0), stop=(p == 15))

                cep_sb = fin_pool.tile([8, 256], FP32, tag="cep_sb")
                nc.scalar.activation(cep_sb[:], cep[0:8, :],
                                     mybir.ActivationFunctionType.Copy)
                for c in range(2):
                    gc = it * 2 + c
                    psC = psC_pool.tile([128, 8], FP32, tag="psC")
                    nc.tensor.transpose(psC[:], cep_sb[:, c * 128:(c + 1) * 128],
                                        identf[0:8, 0:8])
                    mx = fin_pool.tile([128, 1], FP32, tag="mx")
                    nc.vector.reduce_max(mx[:], psC[:], axis=mybir.AxisListType.X)
                    nb = fin_pool.tile([128, 1], FP32, tag="nb")
                    nc.vector.tensor_scalar_mul(nb[:], mx[:], -BETA)
                    ex = fin_pool.tile([128, 8], FP32, tag="ex")
                    nc.scalar.activation(ex[:], psC[:],
                                         mybir.ActivationFunctionType.Exp,
                                         bias=nb[:], scale=BETA)
                    den = fin_pool.tile([128, 1], FP32, tag="den")
                    nc.vector.reduce_sum(den[:], ex[:], axis=mybir.AxisListType.X)
                    en = fin_pool.tile([128, 8], FP32, tag="en")
                    nc.vector.tensor_mul(en[:], ex[:], idxt[:])
                    num = fin_pool.tile([128, 1], FP32, tag="num")
                    nc.vector.reduce_sum(num[:], en[:], axis=mybir.AxisListType.X)
                    rden = fin_pool.tile([128, 1], FP32, tag="rden")
                    nc.vector.reciprocal(rden[:], den[:])
                    mv = fin_pool.tile([128, 1], FP32, tag="mv")
                    nc.vector.tensor_mul(mv[:], num[:], rden[:])
                    symf = fin_pool.tile([128, 1], FP32, tag="symf")
                    nc.vector.tensor_copy(symf[:], symt[:, gc:gc + 1])
                    df = fin_pool.tile([128, 1], FP32, tag="df")
                    nc.vector.tensor_sub(df[:], mv[:], symf[:])
                    ab = fin_pool.tile([128, 1], FP32, tag="ab")
                    nc.scalar.activation(ab[:], df[:],
                                         mybir.ActivationFunctionType.Abs)
                    ls = fin_pool.tile([128, 1], FP32, tag="ls")
                    nc.vector.tensor_scalar_min(ls[:], ab[:], 1.0)
                    nc.sync.dma_start(
                        loss_out[gc * 128:(gc + 1) * 128], ls[:, 0])
    return nc


def kernel(audio_batch, symbols_batch, num_errs_no_reverb_batch,
           num_errs_reverb_batch):
    audio_batch = np.asarray(audio_batch)
    symbols_batch = np.asarray(symbols_batch, dtype=np.int32)
    nn_ = np.asarray(num_errs_no_reverb_batch).astype(np.float32)
    nr_ = np.asarray(num_errs_reverb_batch).astype(np.float32)

    if "nc" not in _cache:
        _cache["nc"] = _install_hoist(_build())
        _cache["tabs"] = _tables()
    nc = _cache["nc"]
    bdcs, h2m, pp, idxt, identf = _cache["tabs"]

    audio_bf = (audio_batch.reshape(B, NW * WIN)
                .astype(ml_dtypes.bfloat16)
                .reshape(NCORES, WLOC, WIN))
    syms = symbols_batch.reshape(NCORES, WLOC)
    in_maps = []
    for c in range(NCORES):
        in_maps.append({
            "audio": audio_bf[c], "syms": syms[c],
            "bdcs": bdcs, "h2m": h2m, "pp": pp,
            "idxt": idxt, "identf": identf,
        })
    import os
    res = run_bass_kernel_spmd(nc, in_maps, core_ids=list(range(NCORES)),
                               trace=bool(os.environ.get("KTRACE")))
    _cache["last_res"] = res
    loss = np.concatenate([res.results[c]["loss_out"] for c in range(NCORES)])
    errs = loss.reshape(B, NW).sum(axis=1, dtype=np.float32)

    tot = np.float32(errs.sum())
    diff = nr_ - nn_
    inv_red = np.where(diff == 0, np.float32(1.0), diff / (nr_ - errs))
    ter = np.float32(inv_red.sum())
    denom = np.float32(B * NW)
    return (np.float32(tot / denom), tot, np.float32(ter / B),
            np.float32(nn_.sum() / denom), np.float32(nr_.sum() / denom))


# revision 19
# speedup vs baseline: 1.9063x; 1.1053x over previous
"""Trainium2 Bass kernel for nn_DecodingLoss (cepstrum decoding loss).

Math (per 4096-sample window):
  cep = irfft(log(|rfft(x)| + eps))[DELAYS]; softargmax(beta=1e10) -> argmax idx;
  loss = clip(|idx - symbol|,0,1); per-audio sums -> 5 scalar outputs.

Kernel strategy (8 cores, pure data parallel over the batch dim):
  FFT 4096 = 32 x 128 Cooley-Tukey: n = 128*t + s  (t<32, s<128)
    stage1 (PE): At[s, (u w4)] = sum_t x[128t+s] W32^{tu} computed with the
      window DATA as the stationary operand and the block-diag C32/S32 as the
      moving operand, so the result lands already corner-turned (no PE
      transposes, no extra PSUM->SBUF round trip).
    evac (DVE/Pool): scatter At re/im from per-group PSUM banks into u-major
      SBUF layout [s, u, w] so stage-2 moving operands are contiguous.
    stage2 (PE): X[k=u+32v] for k=1..2048 via per-u stationary
      [cos|-sin] / [sin|cos] pairs (128 out rows = Xre(v) rows 0:64,
      Xim(v) rows 64:128) streaming 256 contiguous window columns.
  |X|^2: ACT Square (bf16 out) then DVE/Pool partition-folding adds;
  log: ACT Ln; cep taps via one PE projection (f32r); softargmax + loss on
  DVE/ACT. Host sums per-audio errors and mirrors the reference scalar math.
"""
import numpy as np
import ml_dtypes

import concourse.bass as bass
import concourse.mybir as mybir
from concourse import tile
from concourse.bass_utils import run_bass_kernel_spmd

FP32 = mybir.dt.float32
F32R = mybir.dt.float32r
BF16 = mybir.dt.bfloat16
I32 = mybir.dt.int32

B, NW, WIN = 64, 128, 4096
NCORES = 8
BLOC = B // NCORES              # 8 audio rows per core
WLOC = BLOC * NW                # 1024 windows per core
T, S, U = 32, 128, 32           # n = 128 t + s ; k = u + 32 v
NV = 64                         # v-grid size per u
ITERS = 4
WPI = WLOC // ITERS             # 256 windows per iteration
G = WPI // 4                    # 64 groups of 4 windows
DELAYS = np.array([64, 96, 128, 160, 192, 224, 256, 288])
BETA = 1e10

_cache = {}


def _hoist_waits(bir_json):
    """This walrus build rejects instructions carrying attached semaphore waits
    ("Too many sync wait commands"); raw-bass style standalone EventSemaphore
    waits compile and run. Hoist every attached wait into its own
    EventSemaphore on the same engine queue; updates stay attached."""
    import json
    d = json.loads(bir_json)
    n = 0
    for fn in d["functions"]:
        for bb in fn["blocks"]:
            out = []
            for ins in bb["instructions"]:
                si = ins.get("sync_info")
                waits = (si or {}).get("on_wait") or []
                if waits and ins.get("opcode") != "EventSemaphore" and ins.get("engine"):
                    for w in waits:
                        n += 1
                        out.append({
                            "name": f"hoistw-{n}", "opcode": "EventSemaphore",
                            "engine": ins["engine"], "ins": [], "outs": [],
                            "sync_info": {"on_wait": [w], "on_update": []},
                        })
                    si["on_wait"] = []
                out.append(ins)
            bb["instructions"] = out
    return json.dumps(d).encode()


def _install_hoist(nc):
    orig = nc.to_json_bytes
    nc.to_json_bytes = lambda: _hoist_waits(orig())
    return nc
LINEARIZE = False


def _tables():
    t = np.arange(T)[:, None]
    u = np.arange(U)[None, :]
    c32 = np.cos(2 * np.pi * t * u / 32.0)          # [t, u]
    s32n = -np.sin(2 * np.pi * t * u / 32.0)

    # moving operand for swapped stage-1: cols = (cs, u, w4'); rows = (w4, t)
    bdcs = np.zeros((4, T, 2, U, 4), np.float64)
    for w4 in range(4):
        bdcs[w4, :, 0, :, w4] = c32
        bdcs[w4, :, 1, :, w4] = s32n
    bdcs = bdcs.reshape(128, 256)

    # k-grid per u: u==0 -> k = 32*(j+1) (j=0..63), else k = u + 32*j
    kgrid = np.zeros((U, NV), np.int64)
    kgrid[0] = 32 * (np.arange(NV) + 1)
    for uu in range(1, U):
        kgrid[uu] = uu + 32 * np.arange(NV)

    s = np.arange(S)[:, None]
    # stage-2 stationaries: per (u, m) a [S, 128] block; out rows 0:64 = Xre(v),
    # rows 64:128 = Xim(v).  m=0 applies to At_re, m=1 to At_im.
    h2m = np.zeros((S, U, 2, 2, NV), np.float64)
    for uu in range(U):
        ph = 2 * np.pi * s * kgrid[uu][None, :] / 4096.0
        h2m[:, uu, 0, 0] = np.cos(ph)
        h2m[:, uu, 0, 1] = -np.sin(ph)
        h2m[:, uu, 1, 0] = np.sin(ph)
        h2m[:, uu, 1, 1] = np.cos(ph)
    h2m = h2m.reshape(S, U * 2 * 128)

    # projection: cep[d] = sum_k wk*0.5*log(m2)[k]*cos(2 pi k d/4096)/4096
    pp = np.zeros((128, 16, 8), np.float64)
    for p in range(16):
        for half in range(2):
            uu = 2 * p + half
            k = kgrid[uu]
            wk = np.where(k == 2048, 1.0, 2.0)
            for j, d in enumerate(DELAYS):
                pp[half * 64:half * 64 + 64, p, j] = (
                    wk * 0.5 * np.cos(2 * np.pi * k * d / 4096.0) / 4096.0)
    idxt = np.broadcast_to(np.arange(8.0), (128, 8)).copy()
    identf = np.eye(128, dtype=np.float32)
    # PSUM partition fold: out[v] = in[v] + in[64+v]
    sfold = np.zeros((128, 64))
    sfold[np.arange(64), np.arange(64)] = 1.0
    sfold[64 + np.arange(64), np.arange(64)] = 1.0
    return (bdcs.astype(ml_dtypes.bfloat16), h2m.astype(ml_dtypes.bfloat16),
            pp.astype(ml_dtypes.bfloat16), idxt.astype(np.float32), identf,
            sfold.astype(ml_dtypes.bfloat16))


def _build():
    nc = bass.Bass()
    audio = nc.dram_tensor("audio", [WLOC, WIN], BF16, kind="ExternalInput")
    syms = nc.dram_tensor("syms", [WLOC], I32, kind="ExternalInput")
    bdcs_d = nc.dram_tensor("bdcs", [128, 256], BF16, kind="ExternalInput")
    h2m_d = nc.dram_tensor("h2m", [S, U * 2 * 128], BF16, kind="ExternalInput")
    sf_d = nc.dram_tensor("sfold", [128, 64], BF16, kind="ExternalInput")
    pp_d = nc.dram_tensor("pp", [128, 16, 8], BF16, kind="ExternalInput")
    ix_d = nc.dram_tensor("idxt", [128, 8], FP32, kind="ExternalInput")
    idf_d = nc.dram_tensor("identf", [128, 128], FP32, kind="ExternalInput")
    loss_out = nc.dram_tensor("loss_out", [WLOC], FP32, kind="ExternalOutput")

    with tile.TileContext(nc, linearize=LINEARIZE) as tc:
        with (
            tc.tile_pool(name="consts", bufs=1) as consts,
            tc.tile_pool(name="xt", bufs=2) as xt_pool,
            tc.tile_pool(name="at", bufs=2) as at_pool,
            tc.tile_pool(name="sq", bufs=2) as sq_pool,
            tc.tile_pool(name="lg", bufs=2) as lg_pool,
            tc.tile_pool(name="fin", bufs=2) as fin_pool,
            tc.tile_pool(name="ps1", bufs=2, space="PSUM") as ps1_pool,
            tc.tile_pool(name="psX", bufs=3, space="PSUM") as psX_pool,
            tc.tile_pool(name="cep", bufs=1, space="PSUM") as cep_pool,
        ):
            bdcs = consts.tile([128, 256], BF16, tag="bdcs")
            nc.sync.dma_start(bdcs[:], bdcs_d[:])
            idxt = consts.tile([128, 8], FP32, tag="idxt")
            nc.sync.dma_start(idxt[:], ix_d[:])
            identf = consts.tile([128, 128], FP32, tag="identf")
            nc.sync.dma_start(identf[:], idf_d[:])
            h2m = consts.tile([128, U * 2 * 128], BF16, tag="h2m")
            nc.sync.dma_start(h2m[:], h2m_d[:])
            sfold = consts.tile([128, 64], BF16, tag="sfold")
            nc.sync.dma_start(sfold[:], sf_d[:])
            ppj = consts.tile([128, 128], BF16, tag="ppj")
            nc.sync.dma_start(ppj[:], pp_d[:].rearrange("s p j -> s (p j)"))
            epsb = consts.tile([128, 1], FP32, tag="epsb")
            nc.vector.memset(epsb[:], 1e-10)
            symt = consts.tile([128, BLOC], I32, tag="symt")
            nc.sync.dma_start(symt[:], syms[:].rearrange("(c i) -> i c", i=128))

            for it in range(ITERS):
                xt = xt_pool.tile([128, WPI * 32], BF16, tag="xt")
                nc.sync.dma_start(
                    xt[:].rearrange("p (g s) -> p g s", s=S),
                    audio[it * WPI:(it + 1) * WPI, :]
                    .rearrange("(g w4) (t s) -> (w4 t) g s", w4=4, s=S))

                # u-major At layout: cols = u*256 + g*4 + w4
                at_re = at_pool.tile([128, G * 128], BF16, tag="at_re")
                at_im = at_pool.tile([128, G * 128], BF16, tag="at_im")
                atv_re = at_re[:].rearrange("s (u g w) -> s u g w", u=U, w=4)
                atv_im = at_im[:].rearrange("s (u g w) -> s u g w", u=U, w=4)

                for b in range(G // 4):   # 4 window-groups (16 windows) per 2 banks
                    ps1 = ps1_pool.tile([128, 1024], FP32, tag="ps1")
                    for g4 in range(4):
                        nc.tensor.matmul(ps1[:, g4 * 256:(g4 + 1) * 256],
                                         xt[:, (4 * b + g4) * 128:
                                            (4 * b + g4 + 1) * 128],
                                         bdcs[:], start=True, stop=True)
                    # (u, g4, w4) iteration order: dst runs of 16 contiguous
                    # columns instead of scattered 4-element writes
                    ps1v = ps1[:].rearrange("s (g4 c u w) -> s c u g4 w",
                                            g4=4, c=2, w=4)
                    nc.vector.tensor_copy(atv_re[:, :, 4 * b:4 * b + 4, :],
                                          ps1v[:, 0, :, :, :])
                    nc.vector.tensor_copy(atv_im[:, :, 4 * b:4 * b + 4, :],
                                          ps1v[:, 1, :, :, :])

                cep = cep_pool.tile([128, 264], FP32, tag="cep")
                sqs, psMs, lgs = {}, {}, {}

                def s2mm(p):
                    psX = psX_pool.tile([128, 512], FP32, tag="psX")
                    for half in range(2):
                        uu = 2 * p + half
                        off = half * 256
                        rre = at_re[:, uu * 256:(uu + 1) * 256]
                        rim = at_im[:, uu * 256:(uu + 1) * 256]
                        st0 = h2m[:, (uu * 2) * 128:(uu * 2 + 1) * 128]
                        st1 = h2m[:, (uu * 2 + 1) * 128:(uu * 2 + 2) * 128]
                        nc.tensor.matmul(psX[:, off:off + 256], st0, rre,
                                         start=True, stop=False)
                        nc.tensor.matmul(psX[:, off:off + 256], st1, rim,
                                         start=False, stop=True)
                    sq = sq_pool.tile([128, 512], BF16, tag="sq")
                    nc.scalar.activation(sq[:], psX[:],
                                         mybir.ActivationFunctionType.Square)
                    sqs[p] = sq

                def fold(p):
                    # |X|^2 partition fold on the PE: psM[v] = sq[v] + sq[64+v]
                    psM = psX_pool.tile([128, 512], FP32, tag="psX")
                    nc.tensor.matmul(psM[0:64, 0:256], sfold[:],
                                     sqs[p][:, 0:256], start=True, stop=True)
                    nc.tensor.matmul(psM[64:128, 0:256], sfold[:],
                                     sqs[p][:, 256:512], start=True, stop=True)
                    psMs[p] = psM
                    lg = lg_pool.tile([128, 256], BF16, tag="lg")
                    nc.scalar.activation(lg[:], psM[:, 0:256],
                                         mybir.ActivationFunctionType.Ln,
                                         bias=epsb[:])
                    lgs[p] = lg

                def proj(p):
                    nc.tensor.matmul(cep[0:8, 0:256], ppj[:, p * 8:(p + 1) * 8],
                                     lgs[p][:], start=(p == 0), stop=(p == 15))

                for p in range(16):
                    s2mm(p)
                    if p >= 1:
                        fold(p - 1)
                    if p >= 2:
                        proj(p - 2)
                fold(15)
                proj(14)
                proj(15)

                cep_sb = fin_pool.tile([8, 256], FP32, tag="cep_sb")
                nc.vector.tensor_copy(cep_sb[:], cep[0:8, 0:256])
                for c in range(2):
                    gc = it * 2 + c
                    psC = cep[:, 256:264]
                    nc.tensor.transpose(psC, cep_sb[:, c * 128:(c + 1) * 128],
                                        identf[0:8, 0:8])
                    mx = fin_pool.tile([128, 1], FP32, tag="mx")
                    nc.vector.reduce_max(mx[:], psC, axis=mybir.AxisListType.X)
                    nb = fin_pool.tile([128, 1], FP32, tag="nb")
                    nc.vector.tensor_scalar_mul(nb[:], mx[:], -BETA)
                    ex = fin_pool.tile([128, 8], FP32, tag="ex")
                    nc.scalar.activation(ex[:], psC,
                                         mybir.ActivationFunctionType.Exp,
                                         bias=nb[:], scale=BETA)
                    den = fin_pool.tile([128, 1], FP32, tag="den")
                    nc.vector.reduce_sum(den[:], ex[:], axis=mybir.AxisListType.X)
                    en = fin_pool.tile([128, 8], FP32, tag="en")
                    nc.vector.tensor_mul(en[:], ex[:], idxt[:])
                    num = fin_pool.tile([128, 1], FP32, tag="num")
                    nc.vector.reduce_sum(num[:], en[:], axis=mybir.AxisListType.X)
                    rden = fin_pool.tile([128, 1], FP32, tag="rden")
                    nc.vector.reciprocal(rden[:], den[:])
                    mv = fin_pool.tile([128, 1], FP32, tag="mv")
                    nc.vector.tensor_mul(mv[:], num[:], rden[:])
                    symf = fin_pool.tile([128, 1], FP32, tag="symf")
                    nc.vector.tensor_copy(symf[:], symt[:, gc:gc + 1])
                    df = fin_pool.tile([128, 1], FP32, tag="df")
                    nc.vector.tensor_sub(df[:], mv[:], symf[:])
                    ab = fin_pool.tile([128, 1], FP32, tag="ab")
                    nc.scalar.activation(ab[:], df[:],
                                         mybir.ActivationFunctionType.Abs)
                    ls = fin_pool.tile([128, 1], FP32, tag="ls")
                    nc.vector.tensor_scalar_min(ls[:], ab[:], 1.0)
                    nc.sync.dma_start(
                        loss_out[gc * 128:(gc + 1) * 128], ls[:, 0])
    return nc


def kernel(audio_batch, symbols_batch, num_errs_no_reverb_batch,
           num_errs_reverb_batch):
    audio_batch = np.asarray(audio_batch)
    symbols_batch = np.asarray(symbols_batch, dtype=np.int32)
    nn_ = np.asarray(num_errs_no_reverb_batch).astype(np.float32)
    nr_ = np.asarray(num_errs_reverb_batch).astype(np.float32)

    if "nc" not in _cache:
        _cache["nc"] = _install_hoist(_build())
        _cache["tabs"] = _tables()
    nc = _cache["nc"]
    bdcs, h2m, pp, idxt, identf, sfold = _cache["tabs"]

    audio_bf = (audio_batch.reshape(B, NW * WIN)
                .astype(ml_dtypes.bfloat16)
                .reshape(NCORES, WLOC, WIN))
    syms = symbols_batch.reshape(NCORES, WLOC)
    in_maps = []
    for c in range(NCORES):
        in_maps.append({
            "audio": audio_bf[c], "syms": syms[c],
            "bdcs": bdcs, "h2m": h2m, "pp": pp, "sfold": sfold,
            "idxt": idxt, "identf": identf,
        })
    import os
    res = run_bass_kernel_spmd(nc, in_maps, core_ids=list(range(NCORES)),
                               trace=bool(os.environ.get("KTRACE")))
    _cache["last_res"] = res
    loss = np.concatenate([res.results[c]["loss_out"] for c in range(NCORES)])
    errs = loss.reshape(B, NW).sum(axis=1, dtype=np.float32)

    tot = np.float32(errs.sum())
    diff = nr_ - nn_
    inv_red = np.where(diff == 0, np.float32(1.0), diff / (nr_ - errs))
    ter = np.float32(inv_red.sum())
    denom = np.float32(B * NW)
    return (np.float32(tot / denom), tot, np.float32(ter / B),
            np.float32(nn_.sum() / denom), np.float32(nr_.sum() / denom))


# revision 21
# speedup vs baseline: 2.1988x; 1.1534x over previous
"""Trainium2 Bass kernel for nn_DecodingLoss (cepstrum decoding loss).

Math (per 4096-sample window):
  cep = irfft(log(|rfft(x)| + eps))[DELAYS]; softargmax(beta=1e10) -> argmax idx;
  loss = clip(|idx - symbol|,0,1); per-audio sums -> 5 scalar outputs.

Kernel strategy (8 cores, pure data parallel over the batch dim):
  FFT 4096 = 32 x 128 Cooley-Tukey: n = 128*t + s  (t<32, s<128)
    stage1 (PE): At[s, (u w4)] = sum_t x[128t+s] W32^{tu} computed with the
      window DATA as the stationary operand and the block-diag C32/S32 as the
      moving operand, so the result lands already corner-turned (no PE
      transposes, no extra PSUM->SBUF round trip).
    evac (DVE/Pool): scatter At re/im from per-group PSUM banks into u-major
      SBUF layout [s, u, w] so stage-2 moving operands are contiguous.
    stage2 (PE): X[k=u+32v] for k=1..2048 via per-u stationary
      [cos|-sin] / [sin|cos] pairs (128 out rows = Xre(v) rows 0:64,
      Xim(v) rows 64:128) streaming 256 contiguous window columns.
  |X|^2: ACT Square (bf16 out) then DVE/Pool partition-folding adds;
  log: ACT Ln; cep taps via one PE projection (f32r); softargmax + loss on
  DVE/ACT. Host sums per-audio errors and mirrors the reference scalar math.
"""
import numpy as np
import ml_dtypes

import concourse.bass as bass
import concourse.mybir as mybir
from concourse import tile
from concourse.bass_utils import run_bass_kernel_spmd

FP32 = mybir.dt.float32
F32R = mybir.dt.float32r
BF16 = mybir.dt.bfloat16
I32 = mybir.dt.int32

B, NW, WIN = 64, 128, 4096
NCORES = 8
BLOC = B // NCORES              # 8 audio rows per core
WLOC = BLOC * NW                # 1024 windows per core
T, S, U = 32, 128, 32           # n = 128 t + s ; k = u + 32 v
NV = 64                         # v-grid size per u
ITERS = 4
WPI = WLOC // ITERS             # 256 windows per iteration
G = WPI // 4                    # 64 groups of 4 windows
DELAYS = np.array([64, 96, 128, 160, 192, 224, 256, 288])
BETA = 1e10

_cache = {}


def _hoist_waits(bir_json):
    """This walrus build rejects instructions carrying attached semaphore waits
    ("Too many sync wait commands"); raw-bass style standalone EventSemaphore
    waits compile and run. Hoist every attached wait into its own
    EventSemaphore on the same engine queue; updates stay attached."""
    import json
    d = json.loads(bir_json)
    n = 0
    for fn in d["functions"]:
        for bb in fn["blocks"]:
            out = []
            for ins in bb["instructions"]:
                si = ins.get("sync_info")
                waits = (si or {}).get("on_wait") or []
                if waits and ins.get("opcode") != "EventSemaphore" and ins.get("engine"):
                    for w in waits:
                        n += 1
                        out.append({
                            "name": f"hoistw-{n}", "opcode": "EventSemaphore",
                            "engine": ins["engine"], "ins": [], "outs": [],
                            "sync_info": {"on_wait": [w], "on_update": []},
                        })
                    si["on_wait"] = []
                out.append(ins)
            bb["instructions"] = out
    return json.dumps(d).encode()


def _install_hoist(nc):
    orig = nc.to_json_bytes
    nc.to_json_bytes = lambda: _hoist_waits(orig())
    return nc
LINEARIZE = False


def _tables():
    t = np.arange(T)[:, None]
    u = np.arange(U)[None, :]
    c32 = np.cos(2 * np.pi * t * u / 32.0)          # [t, u]
    s32n = -np.sin(2 * np.pi * t * u / 32.0)

    # moving operand for swapped stage-1: cols = (cs, u, w4'); rows = (w4, t)
    bdcs = np.zeros((4, T, 2, U, 4), np.float64)
    for w4 in range(4):
        bdcs[w4, :, 0, :, w4] = c32
        bdcs[w4, :, 1, :, w4] = s32n
    bdcs = bdcs.reshape(128, 256)

    # k-grid per u: u==0 -> k = 32*(j+1) (j=0..63), else k = u + 32*j
    kgrid = np.zeros((U, NV), np.int64)
    kgrid[0] = 32 * (np.arange(NV) + 1)
    for uu in range(1, U):
        kgrid[uu] = uu + 32 * np.arange(NV)

    s = np.arange(S)[:, None]
    # stage-2 stationaries: per (u, m) a [S, 128] block; out rows 0:64 = Xre(v),
    # rows 64:128 = Xim(v).  m=0 applies to At_re, m=1 to At_im.
    h2m = np.zeros((S, U, 2, 2, NV), np.float64)
    for uu in range(U):
        ph = 2 * np.pi * s * kgrid[uu][None, :] / 4096.0
        h2m[:, uu, 0, 0] = np.cos(ph)
        h2m[:, uu, 0, 1] = -np.sin(ph)
        h2m[:, uu, 1, 0] = np.sin(ph)
        h2m[:, uu, 1, 1] = np.cos(ph)
    h2m = h2m.reshape(S, U * 2 * 128)

    # projection: cep[d] = sum_k wk*0.5*log(m2)[k]*cos(2 pi k d/4096)/4096
    pp = np.zeros((128, 16, 8), np.float64)
    for p in range(16):
        for half in range(2):
            uu = 2 * p + half
            k = kgrid[uu]
            wk = np.where(k == 2048, 1.0, 2.0)
            for j, d in enumerate(DELAYS):
                pp[half * 64:half * 64 + 64, p, j] = (
                    wk * 0.5 * np.cos(2 * np.pi * k * d / 4096.0) / 4096.0)
    idxt = np.broadcast_to(np.arange(8.0), (128, 8)).copy()
    identf = np.eye(128, dtype=np.float32)
    # PSUM partition fold: out[v] = in[v] + in[64+v]
    sfold = np.zeros((128, 64))
    sfold[np.arange(64), np.arange(64)] = 1.0
    sfold[64 + np.arange(64), np.arange(64)] = 1.0
    return (bdcs.astype(ml_dtypes.bfloat16), h2m.astype(ml_dtypes.bfloat16),
            pp.astype(ml_dtypes.bfloat16), idxt.astype(np.float32), identf,
            sfold.astype(ml_dtypes.bfloat16))


def _build():
    nc = bass.Bass()
    audio = nc.dram_tensor("audio", [WLOC, WIN], BF16, kind="ExternalInput")
    syms = nc.dram_tensor("syms", [WLOC], I32, kind="ExternalInput")
    bdcs_d = nc.dram_tensor("bdcs", [128, 256], BF16, kind="ExternalInput")
    h2m_d = nc.dram_tensor("h2m", [S, U * 2 * 128], BF16, kind="ExternalInput")
    sf_d = nc.dram_tensor("sfold", [128, 64], BF16, kind="ExternalInput")
    pp_d = nc.dram_tensor("pp", [128, 16, 8], BF16, kind="ExternalInput")
    ix_d = nc.dram_tensor("idxt", [128, 8], FP32, kind="ExternalInput")
    idf_d = nc.dram_tensor("identf", [128, 128], FP32, kind="ExternalInput")
    loss_out = nc.dram_tensor("loss_out", [WLOC], FP32, kind="ExternalOutput")

    with tile.TileContext(nc, linearize=LINEARIZE) as tc:
        with (
            tc.tile_pool(name="consts", bufs=1) as consts,
            tc.tile_pool(name="xt", bufs=2) as xt_pool,
            tc.tile_pool(name="at", bufs=2) as at_pool,
            tc.tile_pool(name="sq", bufs=2) as sq_pool,
            tc.tile_pool(name="lg", bufs=2) as lg_pool,
            tc.tile_pool(name="fin", bufs=2) as fin_pool,
            tc.tile_pool(name="ps1", bufs=2, space="PSUM") as ps1_pool,
            tc.tile_pool(name="psX", bufs=3, space="PSUM") as psX_pool,
            tc.tile_pool(name="cep", bufs=1, space="PSUM") as cep_pool,
        ):
            bdcs = consts.tile([128, 256], BF16, tag="bdcs")
            nc.sync.dma_start(bdcs[:], bdcs_d[:])
            idxt = consts.tile([128, 8], FP32, tag="idxt")
            nc.sync.dma_start(idxt[:], ix_d[:])
            identf = consts.tile([128, 128], FP32, tag="identf")
            nc.sync.dma_start(identf[:], idf_d[:])
            h2m = consts.tile([128, U * 2 * 128], BF16, tag="h2m")
            nc.sync.dma_start(h2m[:], h2m_d[:])
            sfold = consts.tile([128, 64], BF16, tag="sfold")
            nc.sync.dma_start(sfold[:], sf_d[:])
            ppj = consts.tile([128, 128], BF16, tag="ppj")
            nc.sync.dma_start(ppj[:], pp_d[:].rearrange("s p j -> s (p j)"))
            epsb = consts.tile([128, 1], FP32, tag="epsb")
            nc.vector.memset(epsb[:], 1e-10)
            symt = consts.tile([128, BLOC], I32, tag="symt")
            nc.sync.dma_start(symt[:], syms[:].rearrange("(c i) -> i c", i=128))

            xts, ats = {}, {}

            def dma_in(it):
                xt = xt_pool.tile([128, WPI * 32], BF16, tag="xt")
                # 4 chunked transfers so stage-1 starts before the full
                # iteration's audio has landed
                for j in range(4):
                    nc.sync.dma_start(
                        xt[:, j * 2048:(j + 1) * 2048]
                        .rearrange("p (g s) -> p g s", s=S),
                        audio[it * WPI + j * 64:it * WPI + (j + 1) * 64, :]
                        .rearrange("(g w4) (t s) -> (w4 t) g s", w4=4, s=S))
                xts[it] = xt

            def s1_block(it):
                xt = xts[it]
                # u-major At layout: cols = u*256 + g*4 + w4
                at_re = at_pool.tile([128, G * 128], BF16, tag="at_re")
                at_im = at_pool.tile([128, G * 128], BF16, tag="at_im")
                atv_re = at_re[:].rearrange("s (u g w) -> s u g w", u=U, w=4)
                atv_im = at_im[:].rearrange("s (u g w) -> s u g w", u=U, w=4)

                for b in range(G // 4):   # 4 window-groups (16 windows), 2 banks
                    ps1 = ps1_pool.tile([128, 1024], FP32, tag="ps1")
                    for g4 in range(4):
                        nc.tensor.matmul(ps1[:, g4 * 256:(g4 + 1) * 256],
                                         xt[:, (4 * b + g4) * 128:
                                            (4 * b + g4 + 1) * 128],
                                         bdcs[:], start=True, stop=True)
                    # (u, g4, w4) iteration order: dst runs of 16 contiguous
                    # columns instead of scattered 4-element writes
                    ps1v = ps1[:].rearrange("s (g4 c u w) -> s c u g4 w",
                                            g4=4, c=2, w=4)
                    nc.vector.tensor_copy(atv_re[:, :, 4 * b:4 * b + 4, :],
                                          ps1v[:, 0, :, :, :])
                    nc.vector.tensor_copy(atv_im[:, :, 4 * b:4 * b + 4, :],
                                          ps1v[:, 1, :, :, :])
                ats[it] = (at_re, at_im)

            def s2_block(it):
                at_re, at_im = ats.pop(it)
                cep = cep_pool.tile([128, 264], FP32, tag="cep")
                sqs, psMs, lgs = {}, {}, {}

                def s2mm(p):
                    psX = psX_pool.tile([128, 512], FP32, tag="psX")
                    for half in range(2):
                        uu = 2 * p + half
                        off = half * 256
                        rre = at_re[:, uu * 256:(uu + 1) * 256]
                        rim = at_im[:, uu * 256:(uu + 1) * 256]
                        st0 = h2m[:, (uu * 2) * 128:(uu * 2 + 1) * 128]
                        st1 = h2m[:, (uu * 2 + 1) * 128:(uu * 2 + 2) * 128]
                        nc.tensor.matmul(psX[:, off:off + 256], st0, rre,
                                         start=True, stop=False)
                        nc.tensor.matmul(psX[:, off:off + 256], st1, rim,
                                         start=False, stop=True)
                    sq = sq_pool.tile([128, 512], BF16, tag="sq")
                    nc.scalar.activation(sq[:], psX[:],
                                         mybir.ActivationFunctionType.Square)
                    sqs[p] = sq

                def fold(p):
                    # |X|^2 partition fold on the PE: psM[v] = sq[v] + sq[64+v]
                    psM = psX_pool.tile([128, 512], FP32, tag="psX")
                    nc.tensor.matmul(psM[0:64, 0:256], sfold[:],
                                     sqs[p][:, 0:256], start=True, stop=True)
                    nc.tensor.matmul(psM[64:128, 0:256], sfold[:],
                                     sqs[p][:, 256:512], start=True, stop=True)
                    psMs[p] = psM
                    lg = lg_pool.tile([128, 256], BF16, tag="lg")
                    nc.scalar.activation(lg[:], psM[:, 0:256],
                                         mybir.ActivationFunctionType.Ln,
                                         bias=epsb[:])
                    lgs[p] = lg

                def proj(p):
                    nc.tensor.matmul(cep[0:8, 0:256], ppj[:, p * 8:(p + 1) * 8],
                                     lgs[p][:], start=(p == 0), stop=(p == 15))

                for p in range(16):
                    s2mm(p)
                    if p >= 1:
                        fold(p - 1)
                    if p >= 2:
                        proj(p - 2)
                fold(15)
                proj(14)
                proj(15)

                cep_sb = fin_pool.tile([8, 256], FP32, tag="cep_sb")
                nc.vector.tensor_copy(cep_sb[:], cep[0:8, 0:256])
                for c in range(2):
                    gc = it * 2 + c
                    psC = cep[:, 256:264]
                    nc.tensor.transpose(psC, cep_sb[:, c * 128:(c + 1) * 128],
                                        identf[0:8, 0:8])
                    mx = fin_pool.tile([128, 1], FP32, tag="mx")
                    nc.vector.reduce_max(mx[:], psC, axis=mybir.AxisListType.X)
                    nb = fin_pool.tile([128, 1], FP32, tag="nb")
                    nc.vector.tensor_scalar_mul(nb[:], mx[:], -BETA)
                    ex = fin_pool.tile([128, 8], FP32, tag="ex")
                    nc.scalar.activation(ex[:], psC,
                                         mybir.ActivationFunctionType.Exp,
                                         bias=nb[:], scale=BETA)
                    den = fin_pool.tile([128, 1], FP32, tag="den")
                    nc.vector.reduce_sum(den[:], ex[:], axis=mybir.AxisListType.X)
                    en = fin_pool.tile([128, 8], FP32, tag="en")
                    nc.vector.tensor_mul(en[:], ex[:], idxt[:])
                    num = fin_pool.tile([128, 1], FP32, tag="num")
                    nc.vector.reduce_sum(num[:], en[:], axis=mybir.AxisListType.X)
                    rden = fin_pool.tile([128, 1], FP32, tag="rden")
                    nc.vector.reciprocal(rden[:], den[:])
                    mv = fin_pool.tile([128, 1], FP32, tag="mv")
                    nc.vector.tensor_mul(mv[:], num[:], rden[:])
                    symf = fin_pool.tile([128, 1], FP32, tag="symf")
                    nc.vector.tensor_copy(symf[:], symt[:, gc:gc + 1])
                    df = fin_pool.tile([128, 1], FP32, tag="df")
                    nc.vector.tensor_sub(df[:], mv[:], symf[:])
                    ab = fin_pool.tile([128, 1], FP32, tag="ab")
                    nc.scalar.activation(ab[:], df[:],
                                         mybir.ActivationFunctionType.Abs)
                    ls = fin_pool.tile([128, 1], FP32, tag="ls")
                    nc.vector.tensor_scalar_min(ls[:], ab[:], 1.0)
                    nc.sync.dma_start(
                        loss_out[gc * 128:(gc + 1) * 128], ls[:, 0])

            # iteration-level software pipeline: evac(it+1) (DVE) runs while
            # the PE chews on stage-2 of iteration it
            dma_in(0)
            dma_in(1)
            s1_block(0)
            s1_block(1)
            for it in range(ITERS):
                if it + 2 < ITERS:
                    dma_in(it + 2)
                s2_block(it)
                if it + 2 < ITERS:
                    s1_block(it + 2)
    return nc


def kernel(audio_batch, symbols_batch, num_errs_no_reverb_batch,
           num_errs_reverb_batch):
    audio_batch = np.asarray(audio_batch)
    symbols_batch = np.asarray(symbols_batch, dtype=np.int32)
    nn_ = np.asarray(num_errs_no_reverb_batch).astype(np.float32)
    nr_ = np.asarray(num_errs_reverb_batch).astype(np.float32)

    if "nc" not in _cache:
        _cache["nc"] = _install_hoist(_build())
        _cache["tabs"] = _tables()
    nc = _cache["nc"]
    bdcs, h2m, pp, idxt, identf, sfold = _cache["tabs"]

    audio_bf = (audio_batch.reshape(B, NW * WIN)
                .astype(ml_dtypes.bfloat16)
                .reshape(NCORES, WLOC, WIN))
    syms = symbols_batch.reshape(NCORES, WLOC)
    in_maps = []
    for c in range(NCORES):
        in_maps.append({
            "audio": audio_bf[c], "syms": syms[c],
            "bdcs": bdcs, "h2m": h2m, "pp": pp, "sfold": sfold,
            "idxt": idxt, "identf": identf,
        })
    import os
    res = run_bass_kernel_spmd(nc, in_maps, core_ids=list(range(NCORES)),
                               trace=bool(os.environ.get("KTRACE")))
    _cache["last_res"] = res
    loss = np.concatenate([res.results[c]["loss_out"] for c in range(NCORES)])
    errs = loss.reshape(B, NW).sum(axis=1, dtype=np.float32)

    tot = np.float32(errs.sum())
    diff = nr_ - nn_
    inv_red = np.where(diff == 0, np.float32(1.0), diff / (nr_ - errs))
    ter = np.float32(inv_red.sum())
    denom = np.float32(B * NW)
    return (np.float32(tot / denom), tot, np.float32(ter / B),
            np.float32(nn_.sum() / denom), np.float32(nr_.sum() / denom))


# revision 24
# speedup vs baseline: 2.2067x; 1.0036x over previous
"""Trainium2 Bass kernel for nn_DecodingLoss (cepstrum decoding loss).

Math (per 4096-sample window):
  cep = irfft(log(|rfft(x)| + eps))[DELAYS]; softargmax(beta=1e10) -> argmax idx;
  loss = clip(|idx - symbol|,0,1); per-audio sums -> 5 scalar outputs.

Kernel strategy (8 cores, pure data parallel over the batch dim):
  FFT 4096 = 32 x 128 Cooley-Tukey: n = 128*t + s  (t<32, s<128)
    stage1 (PE): At[s, (u w4)] = sum_t x[128t+s] W32^{tu} computed with the
      window DATA as the stationary operand and the block-diag C32/S32 as the
      moving operand, so the result lands already corner-turned (no PE
      transposes, no extra PSUM->SBUF round trip).
    evac (DVE/Pool): scatter At re/im from per-group PSUM banks into u-major
      SBUF layout [s, u, w] so stage-2 moving operands are contiguous.
    stage2 (PE): X[k=u+32v] for k=1..2048 via per-u stationary
      [cos|-sin] / [sin|cos] pairs (128 out rows = Xre(v) rows 0:64,
      Xim(v) rows 64:128) streaming 256 contiguous window columns.
  |X|^2: ACT Square (bf16 out) then DVE/Pool partition-folding adds;
  log: ACT Ln; cep taps via one PE projection (f32r); softargmax + loss on
  DVE/ACT. Host sums per-audio errors and mirrors the reference scalar math.
"""
import numpy as np
import ml_dtypes

import concourse.bass as bass
import concourse.mybir as mybir
from concourse import tile
from concourse.bass_utils import run_bass_kernel_spmd

FP32 = mybir.dt.float32
F32R = mybir.dt.float32r
BF16 = mybir.dt.bfloat16
I32 = mybir.dt.int32

B, NW, WIN = 64, 128, 4096
NCORES = 8
BLOC = B // NCORES              # 8 audio rows per core
WLOC = BLOC * NW                # 1024 windows per core
T, S, U = 32, 128, 32           # n = 128 t + s ; k = u + 32 v
NV = 64                         # v-grid size per u
ITERS = 4
WPI = WLOC // ITERS             # 256 windows per iteration
G = WPI // 4                    # 64 groups of 4 windows
DELAYS = np.array([64, 96, 128, 160, 192, 224, 256, 288])
BETA = 1e10

_cache = {}


def _hoist_waits(bir_json):
    """This walrus build rejects instructions carrying attached semaphore waits
    ("Too many sync wait commands"); raw-bass style standalone EventSemaphore
    waits compile and run. Hoist every attached wait into its own
    EventSemaphore on the same engine queue; updates stay attached."""
    import json
    d = json.loads(bir_json)
    n = 0
    for fn in d["functions"]:
        for bb in fn["blocks"]:
            out = []
            for ins in bb["instructions"]:
                si = ins.get("sync_info")
                waits = (si or {}).get("on_wait") or []
                if waits and ins.get("opcode") != "EventSemaphore" and ins.get("engine"):
                    for w in waits:
                        n += 1
                        out.append({
                            "name": f"hoistw-{n}", "opcode": "EventSemaphore",
                            "engine": ins["engine"], "ins": [], "outs": [],
                            "sync_info": {"on_wait": [w], "on_update": []},
                        })
                    si["on_wait"] = []
                out.append(ins)
            bb["instructions"] = out
    return json.dumps(d).encode()


def _install_hoist(nc):
    orig = nc.to_json_bytes
    nc.to_json_bytes = lambda: _hoist_waits(orig())
    return nc
LINEARIZE = False


def _tables():
    t = np.arange(T)[:, None]
    u = np.arange(U)[None, :]
    c32 = np.cos(2 * np.pi * t * u / 32.0)          # [t, u]
    s32n = -np.sin(2 * np.pi * t * u / 32.0)

    # moving operand for swapped stage-1: cols = (cs, u, w4'); rows = (w4, t)
    bdcs = np.zeros((4, T, 2, U, 4), np.float64)
    for w4 in range(4):
        bdcs[w4, :, 0, :, w4] = c32
        bdcs[w4, :, 1, :, w4] = s32n
    bdcs = bdcs.reshape(128, 256)

    # k-grid per u: u==0 -> k = 32*(j+1) (j=0..63), else k = u + 32*j
    kgrid = np.zeros((U, NV), np.int64)
    kgrid[0] = 32 * (np.arange(NV) + 1)
    for uu in range(1, U):
        kgrid[uu] = uu + 32 * np.arange(NV)

    s = np.arange(S)[:, None]
    # stage-2 stationaries: per (u, m) a [S, 128] block; out rows 0:64 = Xre(v),
    # rows 64:128 = Xim(v).  m=0 applies to At_re, m=1 to At_im.
    h2m = np.zeros((S, U, 2, 2, NV), np.float64)
    for uu in range(U):
        ph = 2 * np.pi * s * kgrid[uu][None, :] / 4096.0
        h2m[:, uu, 0, 0] = np.cos(ph)
        h2m[:, uu, 0, 1] = -np.sin(ph)
        h2m[:, uu, 1, 0] = np.sin(ph)
        h2m[:, uu, 1, 1] = np.cos(ph)
    h2m = h2m.reshape(S, U * 2 * 128)

    # projection: cep[d] = sum_k wk*0.5*log(m2)[k]*cos(2 pi k d/4096)/4096
    pp = np.zeros((128, 16, 8), np.float64)
    for p in range(16):
        for half in range(2):
            uu = 2 * p + half
            k = kgrid[uu]
            wk = np.where(k == 2048, 1.0, 2.0)
            for j, d in enumerate(DELAYS):
                pp[half * 64:half * 64 + 64, p, j] = (
                    wk * 0.5 * np.cos(2 * np.pi * k * d / 4096.0) / 4096.0)
    idxt = np.broadcast_to(np.arange(8.0), (128, 8)).copy()
    identf = np.eye(128, dtype=np.float32)
    # PSUM partition fold: out[v] = in[v] + in[64+v]
    sfold = np.zeros((128, 64))
    sfold[np.arange(64), np.arange(64)] = 1.0
    sfold[64 + np.arange(64), np.arange(64)] = 1.0
    return (bdcs.astype(ml_dtypes.bfloat16), h2m.astype(ml_dtypes.bfloat16),
            pp.astype(ml_dtypes.bfloat16), idxt.astype(np.float32), identf,
            sfold.astype(ml_dtypes.bfloat16))


def _build():
    nc = bass.Bass()
    audio = nc.dram_tensor("audio", [WLOC, WIN], BF16, kind="ExternalInput")
    syms = nc.dram_tensor("syms", [WLOC], I32, kind="ExternalInput")
    bdcs_d = nc.dram_tensor("bdcs", [128, 256], BF16, kind="ExternalInput")
    h2m_d = nc.dram_tensor("h2m", [S, U * 2 * 128], BF16, kind="ExternalInput")
    sf_d = nc.dram_tensor("sfold", [128, 64], BF16, kind="ExternalInput")
    pp_d = nc.dram_tensor("pp", [128, 16, 8], BF16, kind="ExternalInput")
    ix_d = nc.dram_tensor("idxt", [128, 8], FP32, kind="ExternalInput")
    idf_d = nc.dram_tensor("identf", [128, 128], FP32, kind="ExternalInput")
    loss_out = nc.dram_tensor("loss_out", [WLOC], FP32, kind="ExternalOutput")

    with tile.TileContext(nc, linearize=LINEARIZE) as tc:
        with (
            tc.tile_pool(name="consts", bufs=1) as consts,
            tc.tile_pool(name="xt", bufs=2) as xt_pool,
            tc.tile_pool(name="at", bufs=2) as at_pool,
            tc.tile_pool(name="sq", bufs=2) as sq_pool,
            tc.tile_pool(name="lg", bufs=2) as lg_pool,
            tc.tile_pool(name="fin", bufs=2) as fin_pool,
            tc.tile_pool(name="ps1", bufs=2, space="PSUM") as ps1_pool,
            tc.tile_pool(name="psX", bufs=3, space="PSUM") as psX_pool,
            tc.tile_pool(name="cep", bufs=1, space="PSUM") as cep_pool,
        ):
            bdcs = consts.tile([128, 256], BF16, tag="bdcs")
            idxt = consts.tile([128, 8], FP32, tag="idxt")
            identf = consts.tile([128, 128], FP32, tag="identf")
            h2m = consts.tile([128, U * 2 * 128], BF16, tag="h2m")
            sfold = consts.tile([128, 64], BF16, tag="sfold")
            ppj = consts.tile([128, 128], BF16, tag="ppj")
            epsb = consts.tile([128, 1], FP32, tag="epsb")
            nc.vector.memset(epsb[:], 1e-10)
            symt = consts.tile([128, BLOC], I32, tag="symt")

            def load_consts_rest():
                # issued after the first audio chunks: only bdcs gates stage-1
                nc.sync.dma_start(h2m[:], h2m_d[:])
                nc.sync.dma_start(sfold[:], sf_d[:])
                nc.sync.dma_start(ppj[:], pp_d[:].rearrange("s p j -> s (p j)"))
                nc.sync.dma_start(idxt[:], ix_d[:])
                nc.sync.dma_start(identf[:], idf_d[:])
                nc.sync.dma_start(symt[:],
                                  syms[:].rearrange("(c i) -> i c", i=128))

            xts, ats = {}, {}

            def dma_in(it):
                xt = xt_pool.tile([128, WPI * 32], BF16, tag="xt")
                # 4 chunked transfers so stage-1 starts before the full
                # iteration's audio has landed
                for j in range(4):
                    nc.sync.dma_start(
                        xt[:, j * 2048:(j + 1) * 2048]
                        .rearrange("p (g s) -> p g s", s=S),
                        audio[it * WPI + j * 64:it * WPI + (j + 1) * 64, :]
                        .rearrange("(g w4) (t s) -> (w4 t) g s", w4=4, s=S))
                xts[it] = xt

            def s1_block(it):
                xt = xts[it]
                # u-major At layout: cols = u*256 + g*4 + w4
                at_re = at_pool.tile([128, G * 128], BF16, tag="at_re")
                at_im = at_pool.tile([128, G * 128], BF16, tag="at_im")
                atv_re = at_re[:].rearrange("s (u g w) -> s u g w", u=U, w=4)
                atv_im = at_im[:].rearrange("s (u g w) -> s u g w", u=U, w=4)

                for b in range(G // 4):   # 4 window-groups (16 windows), 2 banks
                    ps1 = ps1_pool.tile([128, 1024], FP32, tag="ps1")
                    for g4 in range(4):
                        nc.tensor.matmul(ps1[:, g4 * 256:(g4 + 1) * 256],
                                         xt[:, (4 * b + g4) * 128:
                                            (4 * b + g4 + 1) * 128],
                                         bdcs[:], start=True, stop=True)
                    # (u, g4, w4) iteration order: dst runs of 16 contiguous
                    # columns instead of scattered 4-element writes
                    ps1v = ps1[:].rearrange("s (g4 c u w) -> s c u g4 w",
                                            g4=4, c=2, w=4)
                    nc.vector.tensor_copy(atv_re[:, :, 4 * b:4 * b + 4, :],
                                          ps1v[:, 0, :, :, :])
                    if b % 2 == 0:
                        nc.vector.tensor_copy(atv_im[:, :, 4 * b:4 * b + 4, :],
                                              ps1v[:, 1, :, :, :])
                    else:
                        nc.scalar.activation(atv_im[:, :, 4 * b:4 * b + 4, :],
                                             ps1v[:, 1, :, :, :],
                                             mybir.ActivationFunctionType.Copy)
                ats[it] = (at_re, at_im)

            def s2_block(it):
                at_re, at_im = ats.pop(it)
                cep = cep_pool.tile([128, 264], FP32, tag="cep")
                sqs, psMs, lgs = {}, {}, {}

                def s2mm(p):
                    psX = psX_pool.tile([128, 512], FP32, tag="psX")
                    for half in range(2):
                        uu = 2 * p + half
                        off = half * 256
                        rre = at_re[:, uu * 256:(uu + 1) * 256]
                        rim = at_im[:, uu * 256:(uu + 1) * 256]
                        st0 = h2m[:, (uu * 2) * 128:(uu * 2 + 1) * 128]
                        st1 = h2m[:, (uu * 2 + 1) * 128:(uu * 2 + 2) * 128]
                        nc.tensor.matmul(psX[:, off:off + 256], st0, rre,
                                         start=True, stop=False)
                        nc.tensor.matmul(psX[:, off:off + 256], st1, rim,
                                         start=False, stop=True)
                    sq = sq_pool.tile([128, 512], BF16, tag="sq")
                    nc.scalar.activation(sq[:], psX[:],
                                         mybir.ActivationFunctionType.Square)
                    sqs[p] = sq

                def fold(p):
                    # |X|^2 partition fold on the PE: psM[v] = sq[v] + sq[64+v]
                    psM = psX_pool.tile([128, 512], FP32, tag="psX")
                    nc.tensor.matmul(psM[0:64, 0:256], sfold[:],
                                     sqs[p][:, 0:256], start=True, stop=True)
                    nc.tensor.matmul(psM[64:128, 0:256], sfold[:],
                                     sqs[p][:, 256:512], start=True, stop=True)
                    psMs[p] = psM
                    lg = lg_pool.tile([128, 256], BF16, tag="lg")
                    nc.scalar.activation(lg[:], psM[:, 0:256],
                                         mybir.ActivationFunctionType.Ln,
                                         bias=epsb[:])
                    lgs[p] = lg

                def proj(p):
                    nc.tensor.matmul(cep[0:8, 0:256], ppj[:, p * 8:(p + 1) * 8],
                                     lgs[p][:], start=(p == 0), stop=(p == 15))

                for p in range(16):
                    s2mm(p)
                    if p >= 1:
                        fold(p - 1)
                    if p >= 2:
                        proj(p - 2)
                fold(15)
                proj(14)
                proj(15)

                cep_sb = fin_pool.tile([8, 256], FP32, tag="cep_sb")
                nc.vector.tensor_copy(cep_sb[:], cep[0:8, 0:256])
                for c in range(2):
                    gc = it * 2 + c
                    psC = cep[:, 256:264]
                    nc.tensor.transpose(psC, cep_sb[:, c * 128:(c + 1) * 128],
                                        identf[0:8, 0:8])
                    mx = fin_pool.tile([128, 1], FP32, tag="mx")
                    nc.vector.reduce_max(mx[:], psC, axis=mybir.AxisListType.X)
                    nb = fin_pool.tile([128, 1], FP32, tag="nb")
                    nc.vector.tensor_scalar_mul(nb[:], mx[:], -BETA)
                    ex = fin_pool.tile([128, 8], FP32, tag="ex")
                    nc.scalar.activation(ex[:], psC,
                                         mybir.ActivationFunctionType.Exp,
                                         bias=nb[:], scale=BETA)
                    den = fin_pool.tile([128, 1], FP32, tag="den")
                    nc.vector.reduce_sum(den[:], ex[:], axis=mybir.AxisListType.X)
                    en = fin_pool.tile([128, 8], FP32, tag="en")
                    nc.vector.tensor_mul(en[:], ex[:], idxt[:])
                    num = fin_pool.tile([128, 1], FP32, tag="num")
                    nc.vector.reduce_sum(num[:], en[:], axis=mybir.AxisListType.X)
                    rden = fin_pool.tile([128, 1], FP32, tag="rden")
                    nc.vector.reciprocal(rden[:], den[:])
                    mv = fin_pool.tile([128, 1], FP32, tag="mv")
                    nc.vector.tensor_mul(mv[:], num[:], rden[:])
                    symf = fin_pool.tile([128, 1], FP32, tag="symf")
                    nc.vector.tensor_copy(symf[:], symt[:, gc:gc + 1])
                    df = fin_pool.tile([128, 1], FP32, tag="df")
                    nc.vector.tensor_sub(df[:], mv[:], symf[:])
                    ab = fin_pool.tile([128, 1], FP32, tag="ab")
                    nc.scalar.activation(ab[:], df[:],
                                         mybir.ActivationFunctionType.Abs)
                    ls = fin_pool.tile([128, 1], FP32, tag="ls")
                    nc.vector.tensor_scalar_min(ls[:], ab[:], 1.0)
                    nc.sync.dma_start(
                        loss_out[gc * 128:(gc + 1) * 128], ls[:, 0])

            # iteration-level software pipeline: evac(it+1) (DVE) runs while
            # the PE chews on stage-2 of iteration it
            nc.sync.dma_start(bdcs[:], bdcs_d[:])
            dma_in(0)
            dma_in(1)
            load_consts_rest()
            s1_block(0)
            s1_block(1)
            for it in range(ITERS):
                if it + 2 < ITERS:
                    dma_in(it + 2)
                s2_block(it)
                if it + 2 < ITERS:
                    s1_block(it + 2)
    return nc


def kernel(audio_batch, symbols_batch, num_errs_no_reverb_batch,
           num_errs_reverb_batch):
    audio_batch = np.asarray(audio_batch)
    symbols_batch = np.asarray(symbols_batch, dtype=np.int32)
    nn_ = np.asarray(num_errs_no_reverb_batch).astype(np.float32)
    nr_ = np.asarray(num_errs_reverb_batch).astype(np.float32)

    if "nc" not in _cache:
        _cache["nc"] = _install_hoist(_build())
        _cache["tabs"] = _tables()
    nc = _cache["nc"]
    bdcs, h2m, pp, idxt, identf, sfold = _cache["tabs"]

    audio_bf = (audio_batch.reshape(B, NW * WIN)
                .astype(ml_dtypes.bfloat16)
                .reshape(NCORES, WLOC, WIN))
    syms = symbols_batch.reshape(NCORES, WLOC)
    in_maps = []
    for c in range(NCORES):
        in_maps.append({
            "audio": audio_bf[c], "syms": syms[c],
            "bdcs": bdcs, "h2m": h2m, "pp": pp, "sfold": sfold,
            "idxt": idxt, "identf": identf,
        })
    import os
    res = run_bass_kernel_spmd(nc, in_maps, core_ids=list(range(NCORES)),
                               trace=bool(os.environ.get("KTRACE")))
    _cache["last_res"] = res
    loss = np.concatenate([res.results[c]["loss_out"] for c in range(NCORES)])
    errs = loss.reshape(B, NW).sum(axis=1, dtype=np.float32)

    tot = np.float32(errs.sum())
    diff = nr_ - nn_
    inv_red = np.where(diff == 0, np.float32(1.0), diff / (nr_ - errs))
    ter = np.float32(inv_red.sum())
    denom = np.float32(B * NW)
    return (np.float32(tot / denom), tot, np.float32(ter / B),
            np.float32(nn_.sum() / denom), np.float32(nr_.sum() / denom))
